# revision 10
# baseline (speedup 1.0000x reference)
"""MoE gate (DeepSeek-style noaux_tc routing) Trainium2 kernel, v3.

kernel(**inputs) takes the FULL unsharded inputs
  hidden_states [4, 4096, 7168] f32, weight [256, 7168] f32,
  e_score_correction_bias [256] f32
and returns the FULL outputs (topk_idx [16384, 8] int32,
topk_weight [16384, 8] float32), matching the jax reference.

Sharding: data-parallel over the 16384-token axis across 8 NeuronCores
(2048 tokens each); gate weight + bias replicated.

v3 design:
- Host prep splits x into a bf16 pair (xh = bf16(x), xl = bf16(x-xh))
  and lays it out as [p, chunk, t] so the contraction dim h sits on
  SBUF partitions directly (no PE transposes of x, no device-side
  elementwise prep of x at all). Total x DMA = 2+2 bytes/elem, same
  traffic as the fp32 x. The gate weight ships as a bf16 pair wh/wl
  in [p, chunk, e] layout.
- GEMM computes logitsT[e, t] per 512-token block: stationary = wh/wl
  chunk [128h, 128e] (resident in SBUF, FWL-accelerated LDWEIGHTS
  hidden behind 3 matmuls), moving = xh/xl [128h, 512t] bf16 at
  1 cyc/row. 3-term decomposition
    x @ w ~= xh@wh + xh@wl + xl@wh
  drops only the ~2^-18-relative xl@wl term (bf16 products are exact
  in the fp32 PSUM accumulator), giving rel err ~3e-3 on the top-k
  indices vs the fp32 reference (3 of 16384 tokens near a tie flip).
- logitsT is PE-transposed back (8 x 128x128 fp32 transposes per
  block, ~2k cycles) and routed fully on-chip with the DVE top-8
  instructions (max8 / max_index), an index-matched bias gather, and
  sum-normalization * 2.5.
"""
import sys
sys.path.insert(0, "/opt/trn_rl_repo")
import numpy as np
import ml_dtypes
import concourse.bass as bass
import concourse.tile as tile
from concourse import bacc, mybir

F32 = mybir.dt.float32
BF16 = mybir.dt.bfloat16
U32 = mybir.dt.uint32
I32 = mybir.dt.int32
AF = mybir.ActivationFunctionType
ALU = mybir.AluOpType
AX = mybir.AxisListType
BF16NP = ml_dtypes.bfloat16

H = 7168
E = 256
NG = 8          # expert groups
GS = E // NG    # group size (32)
NCH = H // 128  # 56 h-chunks
G = 14          # chunks per x-DMA group / w slice
NGRP = NCH // G # 4 groups
TB = 512        # token block (psum bank limit: 512 f32)
BIG = 1.0e30


def _build(t_core: int, n_devices: int = 8, repeat: int = 1):
    """in: xh/xl [128, NCH*t_core] bf16 (layout [p, chunk, t]);
        wh/wl [128, NCH*E] bf16 (layout [p, chunk, e]);
        bias_b/iota_b [128, E] f32; ident [128, 128] f32.
    out: idx_out [t_core, 8] i32, w_out [t_core, 8] f32."""
    NB = t_core // TB                   # blocks per core
    TPB = TB // 128                     # 128-token tiles per block
    nc = bacc.Bacc("TRN2", target_bir_lowering=False, debug=False,
                   num_devices=n_devices)

    xh_d = nc.dram_tensor("xh", [128, NCH * t_core], BF16,
                          kind="ExternalInput")
    xl_d = nc.dram_tensor("xl", [128, NCH * t_core], BF16,
                          kind="ExternalInput")
    wh_d = nc.dram_tensor("wh", [128, NCH * E], BF16, kind="ExternalInput")
    wl_d = nc.dram_tensor("wl", [128, NCH * E], BF16, kind="ExternalInput")
    bias_d = nc.dram_tensor("bias_b", [128, E], F32, kind="ExternalInput")
    iota_d = nc.dram_tensor("iota_b", [128, E], F32, kind="ExternalInput")
    ident_d = nc.dram_tensor("ident", [128, 128], F32, kind="ExternalInput")
    idx_d = nc.dram_tensor("idx_out", [t_core, 8], I32, kind="ExternalOutput")
    w_d = nc.dram_tensor("w_out", [t_core, 8], F32, kind="ExternalOutput")

    xh_v = xh_d[:].rearrange("p (c t) -> p c t", t=t_core)
    xl_v = xl_d[:].rearrange("p (c t) -> p c t", t=t_core)
    wh_v = wh_d[:].rearrange("p (c e) -> p c e", e=E)
    wl_v = wl_d[:].rearrange("p (c e) -> p c e", e=E)

    with tile.TileContext(nc) as tc:
        with (
            tc.tile_pool(name="const", bufs=1) as constp,
            tc.tile_pool(name="xin", bufs=2) as xin,
            tc.tile_pool(name="sbt", bufs=4) as sbtp,
            tc.tile_pool(name="route", bufs=3) as rp,
            tc.tile_pool(name="small", bufs=2) as sp,
            tc.tile_pool(name="acc", bufs=4, space="PSUM") as accp,
            tc.tile_pool(name="lpsp", bufs=3, space="PSUM") as lpsp,
            tc.tile_pool(name="warmp", bufs=1, space="PSUM") as warmp,
        ):
            # ---- resident constants ----
            ident = constp.tile([128, 128], F32)
            nc.sync.dma_start(ident[:], ident_d[:])
            # PE warm-up: burn the HAM clock-gate in on the identity
            # while the first x block DMAs (once, outside the bench loop)
            warm = warmp.tile([128, 128], F32, name="warm", tag="warm")
            for _ in range(24):
                nc.tensor.transpose(warm[:], ident[:], ident[:])
            bias_sb = constp.tile([128, E], F32)
            nc.gpsimd.dma_start(bias_sb[:], bias_d[:])
            iota_sb = constp.tile([128, E], F32)
            nc.gpsimd.dma_start(iota_sb[:], iota_d[:])

            # ---- weights: resident bf16, loaded in G-chunk slices on the
            # ACT HWDGE ring (separate FIFO from x loads on the SP ring) ----
            wh_tiles = [constp.tile([128, G, E], BF16, name=f"wh_{s}",
                                    tag=f"wh_{s}") for s in range(NGRP)]
            wl_tiles = [constp.tile([128, G, E], BF16, name=f"wl_{s}",
                                    tag=f"wl_{s}") for s in range(NGRP)]

            def w_load(s):
                nc.scalar.dma_start(wh_tiles[s][:],
                                    wh_v[:, s * G:(s + 1) * G, :])
                nc.scalar.dma_start(wl_tiles[s][:],
                                    wl_v[:, s * G:(s + 1) * G, :])

            def wslice(tiles, c, eh):
                return tiles[c // G][:, c % G, 128 * eh:128 * (eh + 1)]

            def emit_gemm(b):
                """DMA + matmuls for token block b. Returns psum accs."""
                accs = [accp.tile([128, TB], F32, name=f"acc_{b}_{eh}",
                                  tag="acc") for eh in range(2)]
                xhs, xls = {}, {}

                def load(g):
                    xh = xin.tile([128, G, TB], BF16, tag="xh",
                                  name=f"xh_{b}_{g}")
                    nc.sync.dma_start(
                        xh[:], xh_v[:, g * G:(g + 1) * G,
                                    TB * b:TB * (b + 1)])
                    xl = xin.tile([128, G, TB], BF16, tag="xl",
                                  name=f"xl_{b}_{g}")
                    nc.sync.dma_start(
                        xl[:], xl_v[:, g * G:(g + 1) * G,
                                    TB * b:TB * (b + 1)])
                    xhs[g], xls[g] = xh, xl

                if b == 0:
                    w_load(0)
                load(0)
                for g in range(NGRP):
                    if b == 0 and g + 1 < NGRP:
                        w_load(g + 1)
                    if g + 1 < NGRP:
                        load(g + 1)
                    xh, xl = xhs.pop(g), xls.pop(g)
                    for j in range(G):
                        c = g * G + j
                        for eh in range(2):
                            nc.tensor.matmul(
                                accs[eh][:], wslice(wh_tiles, c, eh),
                                xh[:, j, :], start=(c == 0), stop=False)
                            nc.tensor.matmul(
                                accs[eh][:], wslice(wh_tiles, c, eh),
                                xl[:, j, :], start=False, stop=False)
                            nc.tensor.matmul(
                                accs[eh][:], wslice(wl_tiles, c, eh),
                                xh[:, j, :], start=False,
                                stop=(c == NCH - 1))
                return accs

            def emit_back_and_route(b, accs):
                # logitsT [128e, TB] x2  ->  logits [128t, 256e] per tile
                sbts = []
                for eh in range(2):
                    sbt = sbtp.tile([128, TB], F32, tag="sbt",
                                    name=f"sbt_{b}_{eh}")
                    nc.vector.tensor_copy(sbt[:], accs[eh][:])
                    sbts.append(sbt)
                for j4 in range(TPB):
                    i = b * TPB + j4
                    lps = lpsp.tile([128, 256], F32, name=f"lps_{i}",
                                    tag="lps")
                    for eh in range(2):
                        nc.tensor.transpose(
                            lps[:, 128 * eh:128 * (eh + 1)],
                            sbts[eh][:, 128 * j4:128 * (j4 + 1)], ident[:])
                    emit_routing(i, lps)

            def emit_routing(i, lps):
                # sigmoid straight from PSUM: frees the lps bank after one
                # ACT op and skips a logits SBUF copy
                scores = rp.tile([128, E], F32, tag="scores",
                                 name=f"scores_{i}")
                nc.scalar.activation(scores[:], lps[:], AF.Sigmoid)
                sfc = rp.tile([128, E], F32, tag="sfc", name=f"sfc_{i}")
                nc.vector.tensor_tensor(sfc[:], scores[:], bias_sb[:],
                                        op=ALU.add)

                g8 = sp.tile([128, 64], F32, tag="g8", name=f"g8_{i}")
                for g in range(NG):
                    nc.vector.max(g8[:, 8 * g:8 * g + 8],
                                  sfc[:, GS * g:GS * (g + 1)])
                gsc = sp.tile([128, NG], F32, tag="gsc", name=f"gsc_{i}")
                nc.vector.tensor_reduce(
                    gsc[:],
                    g8[:].rearrange("p (g i) -> p g i", i=8)[:, :, 0:2],
                    axis=AX.X, op=ALU.add)

                gt8 = sp.tile([128, 8], F32, tag="gt8", name=f"gt8_{i}")
                nc.vector.max(gt8[:], gsc[:])
                pen = sp.tile([128, NG], F32, tag="pen", name=f"pen_{i}")
                nc.vector.tensor_scalar(pen[:], gsc[:], gt8[:, 3:4], -BIG,
                                        op0=ALU.is_lt, op1=ALU.mult)

                masked = rp.tile([128, E], F32, tag="masked",
                                 name=f"masked_{i}")
                for g in range(NG):
                    nc.gpsimd.tensor_scalar_add(
                        masked[:, GS * g:GS * (g + 1)],
                        sfc[:, GS * g:GS * (g + 1)], pen[:, g:g + 1])

                m8 = sp.tile([128, 8], F32, tag="m8", name=f"m8_{i}")
                nc.vector.max(m8[:], masked[:])
                i8 = sp.tile([128, 8], U32, tag="i8", name=f"i8_{i}")
                nc.vector.max_index(i8[:], m8[:], masked[:])

                # w_raw[k] = m8[k] - bias[i8[k]] (index-matched gather)
                i8f = sp.tile([128, 8], F32, tag="i8f", name=f"i8f_{i}")
                nc.vector.tensor_copy(i8f[:], i8[:])
                junk = rp.tile([128, E], F32, tag="junk", name=f"junk_{i}")
                biasg = sp.tile([128, 8], F32, tag="biasg",
                                name=f"biasg_{i}")
                for k in range(8):
                    eng = nc.vector
                    eng.scalar_tensor_tensor(
                        junk[:], iota_sb[:], i8f[:, k:k + 1], bias_sb[:],
                        op0=ALU.is_equal, op1=ALU.mult,
                        accum_out=biasg[:, k:k + 1])

                wraw = sp.tile([128, 8], F32, tag="wraw", name=f"wraw_{i}")
                nc.vector.tensor_tensor(wraw[:], m8[:], biasg[:],
                                        op=ALU.subtract)
                ssum = sp.tile([128, 1], F32, tag="ssum", name=f"ssum_{i}")
                nc.vector.tensor_reduce(ssum[:], wraw[:], axis=AX.X,
                                        op=ALU.add)
                inv = sp.tile([128, 1], F32, tag="inv", name=f"inv_{i}")
                nc.vector.reciprocal(inv[:], ssum[:])
                wout = sp.tile([128, 8], F32, tag="wout", name=f"wout_{i}")
                nc.vector.tensor_scalar(wout[:], wraw[:], inv[:], 2.5,
                                        op0=ALU.mult, op1=ALU.mult)

                # outputs on the SWDGE ring: keeps them off the ACT ring
                # (sigmoids) and the SP ring (x prefetch), where their wait
                # on the routing chain would head-of-line-block those queues
                nc.gpsimd.dma_start(idx_d[128 * i:128 * (i + 1), :],
                                    i8[:].bitcast(I32))
                nc.gpsimd.dma_start(w_d[128 * i:128 * (i + 1), :], wout[:])

            # defer each block's transpose-back + routing until the next
            # block's GEMM ops are emitted, so the in-order DVE queue
            # never stalls the next block's matmul chain
            def emit_all():
                held = {}
                for b in range(NB):
                    held[b] = emit_gemm(b)
                    if b >= 1:
                        emit_back_and_route(b - 1, held.pop(b - 1))
                emit_back_and_route(NB - 1, held.pop(NB - 1))

            if repeat == 1:
                emit_all()
            else:
                # benchmarking only: loop the whole body on-device
                with tc.For_i(0, repeat, 1):
                    emit_all()

    nc.compile()
    return nc


_NC_CACHE = {}
_T_FULL = 16384
_N_CORES = 8


def make_maps(hidden_states, weight, e_score_correction_bias):
    """Host prep: bf16-pair split + shard + relayout for the 8 cores."""
    t_core = _T_FULL // _N_CORES
    x = np.asarray(hidden_states, dtype=np.float32).reshape(_T_FULL, H)
    w = np.asarray(weight, dtype=np.float32)
    bias = np.asarray(e_score_correction_bias, dtype=np.float32)

    xh = x.astype(BF16NP)
    xl = (x - xh.astype(np.float32)).astype(BF16NP)

    def relayout_x(a):                       # [T, H] -> [core, p, c*t]
        return np.ascontiguousarray(
            a.reshape(_N_CORES, t_core, NCH, 128).transpose(0, 3, 2, 1)
        ).reshape(_N_CORES, 128, NCH * t_core)

    XH, XL = relayout_x(xh), relayout_x(xl)

    wT = np.ascontiguousarray(w.T)           # [H, E]
    wh = wT.astype(BF16NP)
    wl = (wT - wh.astype(np.float32)).astype(BF16NP)

    def relayout_w(a):                       # [H, E] -> [p, c*e]
        return np.ascontiguousarray(
            a.reshape(NCH, 128, E).transpose(1, 0, 2)
        ).reshape(128, NCH * E)

    base = {
        "wh": relayout_w(wh),
        "wl": relayout_w(wl),
        "bias_b": np.ascontiguousarray(
            np.broadcast_to(bias[None, :], (128, E))),
        "iota_b": np.ascontiguousarray(
            np.broadcast_to(np.arange(E, dtype=np.float32)[None, :],
                            (128, E))),
        "ident": np.eye(128, dtype=np.float32),
    }
    maps = []
    for c in range(_N_CORES):
        m = dict(base)
        m["xh"] = XH[c]
        m["xl"] = XL[c]
        maps.append(m)
    return maps


def kernel(hidden_states, weight, e_score_correction_bias):
    from concourse.bass_utils import run_bass_kernel_spmd

    t_core = _T_FULL // _N_CORES
    maps = make_maps(hidden_states, weight, e_score_correction_bias)

    if "v3" not in _NC_CACHE:
        _NC_CACHE["v3"] = _build(t_core, n_devices=_N_CORES)
    nc = _NC_CACHE["v3"]

    br = run_bass_kernel_spmd(nc, maps, list(range(_N_CORES)))
    idx = np.concatenate(
        [br.results[c]["idx_out"] for c in range(_N_CORES)],
        axis=0).astype(np.int32)
    wout = np.concatenate(
        [br.results[c]["w_out"] for c in range(_N_CORES)],
        axis=0).astype(np.float32)
    return idx, wout


# revision 12
# speedup vs baseline: 1.1174x; 1.1174x over previous
"""MoE gate (DeepSeek-style noaux_tc routing) Trainium2 kernel, v3.

kernel(**inputs) takes the FULL unsharded inputs
  hidden_states [4, 4096, 7168] f32, weight [256, 7168] f32,
  e_score_correction_bias [256] f32
and returns the FULL outputs (topk_idx [16384, 8] int32,
topk_weight [16384, 8] float32), matching the jax reference.

Sharding: data-parallel over the 16384-token axis across 8 NeuronCores
(2048 tokens each); gate weight + bias replicated.

v3 design:
- Host prep splits x into a bf16 pair (xh = bf16(x), xl = bf16(x-xh))
  and lays it out as [p, chunk, t] so the contraction dim h sits on
  SBUF partitions directly (no PE transposes of x, no device-side
  elementwise prep of x at all). Total x DMA = 2+2 bytes/elem, same
  traffic as the fp32 x. The gate weight ships as a bf16 pair wh/wl
  in [p, chunk, e] layout.
- GEMM computes logitsT[e, t] per 512-token block: stationary = wh/wl
  chunk [128h, 128e] (resident in SBUF, FWL-accelerated LDWEIGHTS
  hidden behind 3 matmuls), moving = xh/xl [128h, 512t] bf16 at
  1 cyc/row. 3-term decomposition
    x @ w ~= xh@wh + xh@wl + xl@wh
  drops only the ~2^-18-relative xl@wl term (bf16 products are exact
  in the fp32 PSUM accumulator), giving rel err ~3e-3 on the top-k
  indices vs the fp32 reference (3 of 16384 tokens near a tie flip).
- logitsT is PE-transposed back (8 x 128x128 fp32 transposes per
  block, ~2k cycles) and routed fully on-chip with the DVE top-8
  instructions (max8 / max_index), an index-matched bias gather, and
  sum-normalization * 2.5.
"""
import sys
sys.path.insert(0, "/opt/trn_rl_repo")
import numpy as np
import ml_dtypes
import concourse.bass as bass
import concourse.tile as tile
from concourse import bacc, mybir

F32 = mybir.dt.float32
BF16 = mybir.dt.bfloat16
U32 = mybir.dt.uint32
I32 = mybir.dt.int32
AF = mybir.ActivationFunctionType
ALU = mybir.AluOpType
AX = mybir.AxisListType
BF16NP = ml_dtypes.bfloat16

H = 7168
E = 256
NG = 8          # expert groups
GS = E // NG    # group size (32)
NCH = H // 128  # 56 h-chunks
G = 14          # chunks per x-DMA group / w slice
NGRP = NCH // G # 4 groups
TB = 512        # token block (psum bank limit: 512 f32)
BIG = 1.0e30


def _build(t_core: int, n_devices: int = 8, repeat: int = 1):
    """in: xh/xl [128, NCH*t_core] bf16 (layout [p, chunk, t]);
        wh/wl [128, NCH*E] bf16 (layout [p, chunk, e]);
        bias_b/iota_b [128, E] f32; ident [128, 128] f32.
    out: idx_out [t_core, 8] i32, w_out [t_core, 8] f32."""
    NB = t_core // TB                   # blocks per core
    TPB = TB // 128                     # 128-token tiles per block
    nc = bacc.Bacc("TRN2", target_bir_lowering=False, debug=False,
                   num_devices=n_devices)

    xh_d = nc.dram_tensor("xh", [128, NCH * t_core], BF16,
                          kind="ExternalInput")
    xl_d = nc.dram_tensor("xl", [128, NCH * t_core], BF16,
                          kind="ExternalInput")
    wh_d = nc.dram_tensor("wh", [128, NCH * E], BF16, kind="ExternalInput")
    wl_d = nc.dram_tensor("wl", [128, NCH * E], BF16, kind="ExternalInput")
    bias_d = nc.dram_tensor("bias_b", [128, E], F32, kind="ExternalInput")
    iota_d = nc.dram_tensor("iota_b", [128, E], F32, kind="ExternalInput")
    ident_d = nc.dram_tensor("ident", [128, 128], F32, kind="ExternalInput")
    idx_d = nc.dram_tensor("idx_out", [t_core, 8], I32, kind="ExternalOutput")
    w_d = nc.dram_tensor("w_out", [t_core, 8], F32, kind="ExternalOutput")

    xh_v = xh_d[:].rearrange("p (c t) -> p c t", t=t_core)
    xl_v = xl_d[:].rearrange("p (c t) -> p c t", t=t_core)
    wh_v = wh_d[:].rearrange("p (c e) -> p c e", e=E)
    wl_v = wl_d[:].rearrange("p (c e) -> p c e", e=E)

    with tile.TileContext(nc) as tc:
        with (
            tc.tile_pool(name="const", bufs=1) as constp,
            tc.tile_pool(name="xin", bufs=2) as xin,
            tc.tile_pool(name="sbt", bufs=4) as sbtp,
            tc.tile_pool(name="route", bufs=5) as rp,
            tc.tile_pool(name="small", bufs=5) as sp,
            tc.tile_pool(name="acc", bufs=4, space="PSUM") as accp,
            tc.tile_pool(name="lpsp", bufs=3, space="PSUM") as lpsp,
            tc.tile_pool(name="warmp", bufs=1, space="PSUM") as warmp,
        ):
            # ---- resident constants ----
            ident = constp.tile([128, 128], F32)
            nc.sync.dma_start(ident[:], ident_d[:])
            # PE warm-up: burn the HAM clock-gate in on the identity
            # while the first x block DMAs (once, outside the bench loop)
            warm = warmp.tile([128, 128], F32, name="warm", tag="warm")
            for _ in range(24):
                nc.tensor.transpose(warm[:], ident[:], ident[:])
            bias_sb = constp.tile([128, E], F32)
            nc.gpsimd.dma_start(bias_sb[:], bias_d[:])
            iota_sb = constp.tile([128, E], F32)
            nc.gpsimd.dma_start(iota_sb[:], iota_d[:])

            # ---- weights: resident bf16, loaded in G-chunk slices on the
            # ACT HWDGE ring (separate FIFO from x loads on the SP ring) ----
            wh_tiles = [constp.tile([128, G, E], BF16, name=f"wh_{s}",
                                    tag=f"wh_{s}") for s in range(NGRP)]
            wl_tiles = [constp.tile([128, G, E], BF16, name=f"wl_{s}",
                                    tag=f"wl_{s}") for s in range(NGRP)]

            def w_load(s):
                nc.scalar.dma_start(wh_tiles[s][:],
                                    wh_v[:, s * G:(s + 1) * G, :])
                nc.scalar.dma_start(wl_tiles[s][:],
                                    wl_v[:, s * G:(s + 1) * G, :])

            def wslice(tiles, c, eh):
                return tiles[c // G][:, c % G, 128 * eh:128 * (eh + 1)]

            def emit_gemm(b):
                """DMA + matmuls for token block b. Returns psum accs."""
                accs = [accp.tile([128, TB], F32, name=f"acc_{b}_{eh}",
                                  tag="acc") for eh in range(2)]
                xhs, xls = {}, {}

                def load(g):
                    xh = xin.tile([128, G, TB], BF16, tag="xh",
                                  name=f"xh_{b}_{g}")
                    nc.sync.dma_start(
                        xh[:], xh_v[:, g * G:(g + 1) * G,
                                    TB * b:TB * (b + 1)])
                    xl = xin.tile([128, G, TB], BF16, tag="xl",
                                  name=f"xl_{b}_{g}")
                    nc.sync.dma_start(
                        xl[:], xl_v[:, g * G:(g + 1) * G,
                                    TB * b:TB * (b + 1)])
                    xhs[g], xls[g] = xh, xl

                if b == 0:
                    w_load(0)
                load(0)
                for g in range(NGRP):
                    if b == 0 and g + 1 < NGRP:
                        w_load(g + 1)
                    if g + 1 < NGRP:
                        load(g + 1)
                    xh, xl = xhs.pop(g), xls.pop(g)
                    for j in range(G):
                        c = g * G + j
                        for eh in range(2):
                            nc.tensor.matmul(
                                accs[eh][:], wslice(wh_tiles, c, eh),
                                xh[:, j, :], start=(c == 0), stop=False)
                            nc.tensor.matmul(
                                accs[eh][:], wslice(wh_tiles, c, eh),
                                xl[:, j, :], start=False, stop=False)
                            nc.tensor.matmul(
                                accs[eh][:], wslice(wl_tiles, c, eh),
                                xh[:, j, :], start=False,
                                stop=(c == NCH - 1))
                return accs

            def emit_back(b, accs):
                # logitsT [128e, TB] x2 -> logits [128t, 256e] per tile,
                # then sigmoid straight from PSUM (frees the lps bank after
                # one ACT op; ACT is otherwise idle)
                sbts = []
                for eh in range(2):
                    sbt = sbtp.tile([128, TB], F32, tag="sbt",
                                    name=f"sbt_{b}_{eh}")
                    nc.vector.tensor_copy(sbt[:], accs[eh][:])
                    sbts.append(sbt)
                scs = []
                for j4 in range(TPB):
                    i = b * TPB + j4
                    lps = lpsp.tile([128, 256], F32, name=f"lps_{i}",
                                    tag="lps")
                    for eh in range(2):
                        nc.tensor.transpose(
                            lps[:, 128 * eh:128 * (eh + 1)],
                            sbts[eh][:, 128 * j4:128 * (j4 + 1)], ident[:])
                    scores = rp.tile([128, E], F32, tag="scores",
                                     name=f"scores_{i}", bufs=2 * TPB + 1)
                    nc.scalar.activation(scores[:], lps[:], AF.Sigmoid)
                    scs.append(scores)
                return scs

            def emit_route_block(b, scs):
                # software-pipelined routing for the TPB tiles of block b:
                # every stage is emitted for all tiles back-to-back so the
                # in-order DVE queue overlaps one tile's dependency gaps
                # with the other tiles' work
                ts = list(range(TPB))
                ii = [b * TPB + j for j in ts]

                def tiles(pool, shape, tag, n=TPB):
                    return [pool.tile(shape, F32, tag=tag,
                                      name=f"{tag}_{ii[j]}")
                            for j in range(n)]

                sfc = tiles(rp, [128, E], "sfc")
                for j in ts:
                    nc.vector.tensor_tensor(sfc[j][:], scs[j][:],
                                            bias_sb[:], op=ALU.add)
                g8 = tiles(sp, [128, 64], "g8")
                for j in ts:
                    for g in range(NG):
                        nc.vector.max(g8[j][:, 8 * g:8 * g + 8],
                                      sfc[j][:, GS * g:GS * (g + 1)])
                gsc = tiles(sp, [128, NG], "gsc")
                for j in ts:
                    nc.vector.tensor_reduce(
                        gsc[j][:],
                        g8[j][:].rearrange("p (g i) -> p g i", i=8)[:, :, 0:2],
                        axis=AX.X, op=ALU.add)
                gt8 = tiles(sp, [128, 8], "gt8")
                for j in ts:
                    nc.vector.max(gt8[j][:], gsc[j][:])
                pen = tiles(sp, [128, NG], "pen")
                for j in ts:
                    nc.vector.tensor_scalar(pen[j][:], gsc[j][:],
                                            gt8[j][:, 3:4], -BIG,
                                            op0=ALU.is_lt, op1=ALU.mult)
                masked = tiles(rp, [128, E], "masked")
                for j in ts:
                    for g in range(NG):
                        nc.vector.tensor_scalar_add(
                            masked[j][:, GS * g:GS * (g + 1)],
                            sfc[j][:, GS * g:GS * (g + 1)],
                            pen[j][:, g:g + 1])
                m8 = tiles(sp, [128, 8], "m8")
                for j in ts:
                    nc.vector.max(m8[j][:], masked[j][:])
                i8 = [sp.tile([128, 8], U32, tag="i8", name=f"i8_{ii[j]}")
                      for j in ts]
                for j in ts:
                    nc.vector.max_index(i8[j][:], m8[j][:], masked[j][:])
                i8f = tiles(sp, [128, 8], "i8f")
                for j in ts:
                    nc.vector.tensor_copy(i8f[j][:], i8[j][:])
                # w_raw[k] = m8[k] - bias[i8[k]] (index-matched gather)
                junk = tiles(rp, [128, E], "junk", n=1)
                biasg = tiles(sp, [128, 8], "biasg")
                for j in ts:
                    for k in range(8):
                        nc.vector.scalar_tensor_tensor(
                            junk[0][:], iota_sb[:], i8f[j][:, k:k + 1],
                            bias_sb[:], op0=ALU.is_equal, op1=ALU.mult,
                            accum_out=biasg[j][:, k:k + 1])
                wraw = tiles(sp, [128, 8], "wraw")
                for j in ts:
                    nc.vector.tensor_tensor(wraw[j][:], m8[j][:],
                                            biasg[j][:], op=ALU.subtract)
                ssum = tiles(sp, [128, 1], "ssum")
                for j in ts:
                    nc.vector.tensor_reduce(ssum[j][:], wraw[j][:],
                                            axis=AX.X, op=ALU.add)
                inv = tiles(sp, [128, 1], "inv")
                for j in ts:
                    nc.vector.reciprocal(inv[j][:], ssum[j][:])
                wout = tiles(sp, [128, 8], "wout")
                for j in ts:
                    nc.vector.tensor_scalar(wout[j][:], wraw[j][:],
                                            inv[j][:], 2.5,
                                            op0=ALU.mult, op1=ALU.mult)
                # outputs on the SWDGE ring: keeps their routing-chain wait
                # off the ACT ring (sigmoids) and the SP ring (x prefetch)
                for j in ts:
                    i = ii[j]
                    nc.gpsimd.dma_start(idx_d[128 * i:128 * (i + 1), :],
                                        i8[j][:].bitcast(I32))
                    nc.gpsimd.dma_start(w_d[128 * i:128 * (i + 1), :],
                                        wout[j][:])

            # transpose-back + sigmoid immediately after each block's GEMM
            # (DVE is free then); defer only the DVE routing stages past the
            # next block's matmuls so they overlap the next GEMM
            def emit_all():
                held = {}
                for b in range(NB):
                    accs = emit_gemm(b)
                    held[b] = emit_back(b, accs)
                    if b >= 1:
                        emit_route_block(b - 1, held.pop(b - 1))
                emit_route_block(NB - 1, held.pop(NB - 1))

            if repeat == 1:
                emit_all()
            else:
                # benchmarking only: loop the whole body on-device
                with tc.For_i(0, repeat, 1):
                    emit_all()

    nc.compile()
    return nc


_NC_CACHE = {}
_T_FULL = 16384
_N_CORES = 8


def make_maps(hidden_states, weight, e_score_correction_bias):
    """Host prep: bf16-pair split + shard + relayout for the 8 cores."""
    t_core = _T_FULL // _N_CORES
    x = np.asarray(hidden_states, dtype=np.float32).reshape(_T_FULL, H)
    w = np.asarray(weight, dtype=np.float32)
    bias = np.asarray(e_score_correction_bias, dtype=np.float32)

    xh = x.astype(BF16NP)
    xl = (x - xh.astype(np.float32)).astype(BF16NP)

    def relayout_x(a):                       # [T, H] -> [core, p, c*t]
        return np.ascontiguousarray(
            a.reshape(_N_CORES, t_core, NCH, 128).transpose(0, 3, 2, 1)
        ).reshape(_N_CORES, 128, NCH * t_core)

    XH, XL = relayout_x(xh), relayout_x(xl)

    wT = np.ascontiguousarray(w.T)           # [H, E]
    wh = wT.astype(BF16NP)
    wl = (wT - wh.astype(np.float32)).astype(BF16NP)

    def relayout_w(a):                       # [H, E] -> [p, c*e]
        return np.ascontiguousarray(
            a.reshape(NCH, 128, E).transpose(1, 0, 2)
        ).reshape(128, NCH * E)

    base = {
        "wh": relayout_w(wh),
        "wl": relayout_w(wl),
        "bias_b": np.ascontiguousarray(
            np.broadcast_to(bias[None, :], (128, E))),
        "iota_b": np.ascontiguousarray(
            np.broadcast_to(np.arange(E, dtype=np.float32)[None, :],
                            (128, E))),
        "ident": np.eye(128, dtype=np.float32),
    }
    maps = []
    for c in range(_N_CORES):
        m = dict(base)
        m["xh"] = XH[c]
        m["xl"] = XL[c]
        maps.append(m)
    return maps


def kernel(hidden_states, weight, e_score_correction_bias):
    from concourse.bass_utils import run_bass_kernel_spmd

    t_core = _T_FULL // _N_CORES
    maps = make_maps(hidden_states, weight, e_score_correction_bias)

    if "v3" not in _NC_CACHE:
        _NC_CACHE["v3"] = _build(t_core, n_devices=_N_CORES)
    nc = _NC_CACHE["v3"]

    br = run_bass_kernel_spmd(nc, maps, list(range(_N_CORES)))
    idx = np.concatenate(
        [br.results[c]["idx_out"] for c in range(_N_CORES)],
        axis=0).astype(np.int32)
    wout = np.concatenate(
        [br.results[c]["w_out"] for c in range(_N_CORES)],
        axis=0).astype(np.float32)
    return idx, wout


# revision 15
# speedup vs baseline: 1.1260x; 1.0077x over previous
"""MoE gate (DeepSeek-style noaux_tc routing) Trainium2 kernel, v3.

kernel(**inputs) takes the FULL unsharded inputs
  hidden_states [4, 4096, 7168] f32, weight [256, 7168] f32,
  e_score_correction_bias [256] f32
and returns the FULL outputs (topk_idx [16384, 8] int32,
topk_weight [16384, 8] float32), matching the jax reference.

Sharding: data-parallel over the 16384-token axis across 8 NeuronCores
(2048 tokens each); gate weight + bias replicated.

v3 design:
- Host prep splits x into a bf16 pair (xh = bf16(x), xl = bf16(x-xh))
  and lays it out as [p, chunk, t] so the contraction dim h sits on
  SBUF partitions directly (no PE transposes of x, no device-side
  elementwise prep of x at all). Total x DMA = 2+2 bytes/elem, same
  traffic as the fp32 x. The gate weight ships as a bf16 pair wh/wl
  in [p, chunk, e] layout.
- GEMM computes logitsT[e, t] per 512-token block: stationary = wh/wl
  chunk [128h, 128e] (resident in SBUF, FWL-accelerated LDWEIGHTS
  hidden behind 3 matmuls), moving = xh/xl [128h, 512t] bf16 at
  1 cyc/row. 3-term decomposition
    x @ w ~= xh@wh + xh@wl + xl@wh
  drops only the ~2^-18-relative xl@wl term (bf16 products are exact
  in the fp32 PSUM accumulator), giving rel err ~3e-3 on the top-k
  indices vs the fp32 reference (3 of 16384 tokens near a tie flip).
- logitsT is PE-transposed back (8 x 128x128 fp32 transposes per
  block, ~2k cycles) and routed fully on-chip with the DVE top-8
  instructions (max8 / max_index), an index-matched bias gather, and
  sum-normalization * 2.5.
"""
import sys
sys.path.insert(0, "/opt/trn_rl_repo")
import numpy as np
import ml_dtypes
import concourse.bass as bass
import concourse.tile as tile
from concourse import bacc, mybir

F32 = mybir.dt.float32
BF16 = mybir.dt.bfloat16
U32 = mybir.dt.uint32
I32 = mybir.dt.int32
AF = mybir.ActivationFunctionType
ALU = mybir.AluOpType
AX = mybir.AxisListType
BF16NP = ml_dtypes.bfloat16

H = 7168
E = 256
NG = 8          # expert groups
GS = E // NG    # group size (32)
NCH = H // 128  # 56 h-chunks
G = 14          # chunks per x-DMA group / w slice
NGRP = NCH // G # 4 groups
TB = 512        # token block (psum bank limit: 512 f32)
BIG = 1.0e30


def _build(t_core: int, n_devices: int = 8, repeat: int = 1):
    """in: xh/xl [128, NCH*t_core] bf16 (layout [p, chunk, t]);
        wh/wl [128, NCH*E] bf16 (layout [p, chunk, e]);
        bias_b/iota_b [128, E] f32; ident [128, 128] f32.
    out: idx_out [t_core, 8] i32, w_out [t_core, 8] f32."""
    NB = t_core // TB                   # blocks per core
    TPB = TB // 128                     # 128-token tiles per block
    nc = bacc.Bacc("TRN2", target_bir_lowering=False, debug=False,
                   num_devices=n_devices)

    xh_d = nc.dram_tensor("xh", [128, NCH * t_core], BF16,
                          kind="ExternalInput")
    xl_d = nc.dram_tensor("xl", [128, NCH * t_core], BF16,
                          kind="ExternalInput")
    wh_d = nc.dram_tensor("wh", [128, NCH * E], BF16, kind="ExternalInput")
    wl_d = nc.dram_tensor("wl", [128, NCH * E], BF16, kind="ExternalInput")
    bias_d = nc.dram_tensor("bias_b", [128, E], F32, kind="ExternalInput")
    iota_d = nc.dram_tensor("iota_b", [128, E], F32, kind="ExternalInput")
    ident_d = nc.dram_tensor("ident", [128, 128], F32, kind="ExternalInput")
    idx_d = nc.dram_tensor("idx_out", [t_core, 8], I32, kind="ExternalOutput")
    w_d = nc.dram_tensor("w_out", [t_core, 8], F32, kind="ExternalOutput")

    xh_v = xh_d[:].rearrange("p (c t) -> p c t", t=t_core)
    xl_v = xl_d[:].rearrange("p (c t) -> p c t", t=t_core)
    wh_v = wh_d[:].rearrange("p (c e) -> p c e", e=E)
    wl_v = wl_d[:].rearrange("p (c e) -> p c e", e=E)

    with tile.TileContext(nc) as tc:
        with (
            tc.tile_pool(name="const", bufs=1) as constp,
            tc.tile_pool(name="xin", bufs=2) as xin,
            tc.tile_pool(name="sbt", bufs=4) as sbtp,
            tc.tile_pool(name="route", bufs=5) as rp,
            tc.tile_pool(name="small", bufs=5) as sp,
            tc.tile_pool(name="acc", bufs=4, space="PSUM") as accp,
            tc.tile_pool(name="lpsp", bufs=3, space="PSUM") as lpsp,
            tc.tile_pool(name="warmp", bufs=1, space="PSUM") as warmp,
        ):
            # ---- resident constants ----
            ident = constp.tile([128, 128], F32)
            nc.sync.dma_start(ident[:], ident_d[:])
            # PE warm-up: burn the HAM clock-gate in on the identity
            # while the first x block DMAs (once, outside the bench loop)
            warm = warmp.tile([128, 128], F32, name="warm", tag="warm")
            for _ in range(24):
                nc.tensor.transpose(warm[:], ident[:], ident[:])
            bias_sb = constp.tile([128, E], F32)
            nc.gpsimd.dma_start(bias_sb[:], bias_d[:])
            iota_sb = constp.tile([128, E], F32)
            nc.gpsimd.dma_start(iota_sb[:], iota_d[:])

            # ---- weights: resident bf16, loaded in G-chunk slices on the
            # ACT HWDGE ring (separate FIFO from x loads on the SP ring) ----
            wh_tiles = [constp.tile([128, G, E], BF16, name=f"wh_{s}",
                                    tag=f"wh_{s}") for s in range(NGRP)]
            wl_tiles = [constp.tile([128, G, E], BF16, name=f"wl_{s}",
                                    tag=f"wl_{s}") for s in range(NGRP)]

            def w_load(s):
                nc.scalar.dma_start(wh_tiles[s][:],
                                    wh_v[:, s * G:(s + 1) * G, :])
                nc.scalar.dma_start(wl_tiles[s][:],
                                    wl_v[:, s * G:(s + 1) * G, :])

            def wslice(tiles, c, eh):
                return tiles[c // G][:, c % G, 128 * eh:128 * (eh + 1)]

            def emit_gemm(b):
                """DMA + matmuls for token block b. Returns psum accs."""
                accs = [accp.tile([128, TB], F32, name=f"acc_{b}_{eh}",
                                  tag="acc") for eh in range(2)]
                xhs, xls = {}, {}

                def load(g):
                    xh = xin.tile([128, G, TB], BF16, tag="xh",
                                  name=f"xh_{b}_{g}")
                    nc.sync.dma_start(
                        xh[:], xh_v[:, g * G:(g + 1) * G,
                                    TB * b:TB * (b + 1)])
                    xl = xin.tile([128, G, TB], BF16, tag="xl",
                                  name=f"xl_{b}_{g}")
                    nc.sync.dma_start(
                        xl[:], xl_v[:, g * G:(g + 1) * G,
                                    TB * b:TB * (b + 1)])
                    xhs[g], xls[g] = xh, xl

                if b == 0:
                    w_load(0)
                load(0)
                for g in range(NGRP):
                    if b == 0 and g + 1 < NGRP:
                        w_load(g + 1)
                    if g + 1 < NGRP:
                        load(g + 1)
                    xh, xl = xhs.pop(g), xls.pop(g)
                    for j in range(G):
                        c = g * G + j
                        for eh in range(2):
                            nc.tensor.matmul(
                                accs[eh][:], wslice(wh_tiles, c, eh),
                                xh[:, j, :], start=(c == 0), stop=False)
                            nc.tensor.matmul(
                                accs[eh][:], wslice(wh_tiles, c, eh),
                                xl[:, j, :], start=False, stop=False)
                            nc.tensor.matmul(
                                accs[eh][:], wslice(wl_tiles, c, eh),
                                xh[:, j, :], start=False,
                                stop=(c == NCH - 1))
                return accs

            def emit_back(b, accs):
                # logitsT [128e, TB] x2 -> logits [128t, 256e] per tile,
                # then sigmoid straight from PSUM (frees the lps bank after
                # one ACT op; ACT is otherwise idle)
                sbts = []
                for eh in range(2):
                    sbt = sbtp.tile([128, TB], F32, tag="sbt",
                                    name=f"sbt_{b}_{eh}")
                    # on ACT: keeps the acc-stop wait out of the DVE queue,
                    # which runs pure routing work
                    nc.scalar.copy(sbt[:], accs[eh][:])
                    sbts.append(sbt)
                scs = []
                for j4 in range(TPB):
                    i = b * TPB + j4
                    lps = lpsp.tile([128, 256], F32, name=f"lps_{i}",
                                    tag="lps")
                    for eh in range(2):
                        nc.tensor.transpose(
                            lps[:, 128 * eh:128 * (eh + 1)],
                            sbts[eh][:, 128 * j4:128 * (j4 + 1)], ident[:])
                    scores = rp.tile([128, E], F32, tag="scores",
                                     name=f"scores_{i}", bufs=2 * TPB + 1)
                    nc.scalar.activation(scores[:], lps[:], AF.Sigmoid)
                    scs.append(scores)
                return scs

            def emit_route_block(b, scs):
                # software-pipelined routing for the TPB tiles of block b:
                # every stage is emitted for all tiles back-to-back so the
                # in-order DVE queue overlaps one tile's dependency gaps
                # with the other tiles' work
                ts = list(range(TPB))
                ii = [b * TPB + j for j in ts]

                def tiles(pool, shape, tag, n=TPB):
                    return [pool.tile(shape, F32, tag=tag,
                                      name=f"{tag}_{ii[j]}")
                            for j in range(n)]

                sfc = tiles(rp, [128, E], "sfc")
                for j in ts:
                    nc.vector.tensor_tensor(sfc[j][:], scs[j][:],
                                            bias_sb[:], op=ALU.add)
                # group top-2 sum via reduce-max + match_replace + reduce-max
                gt1 = tiles(sp, [128, NG], "gt1")
                for j in ts:
                    nc.vector.tensor_reduce(
                        gt1[j][:],
                        sfc[j][:].rearrange("p (g i) -> p g i", i=GS),
                        axis=AX.X, op=ALU.max)
                sfc2 = tiles(rp, [128, E], "sfc2")
                for j in ts:
                    nc.vector.match_replace(sfc2[j][:], gt1[j][:],
                                            sfc[j][:], -BIG)
                gt2 = tiles(sp, [128, NG], "gt2")
                for j in ts:
                    nc.vector.tensor_reduce(
                        gt2[j][:],
                        sfc2[j][:].rearrange("p (g i) -> p g i", i=GS),
                        axis=AX.X, op=ALU.max)
                gsc = tiles(sp, [128, NG], "gsc")
                for j in ts:
                    nc.vector.tensor_tensor(gsc[j][:], gt1[j][:],
                                            gt2[j][:], op=ALU.add)
                gt8 = tiles(sp, [128, 8], "gt8")
                for j in ts:
                    nc.vector.max(gt8[j][:], gsc[j][:])
                pen = tiles(sp, [128, NG], "pen")
                for j in ts:
                    nc.vector.tensor_scalar(pen[j][:], gsc[j][:],
                                            gt8[j][:, 3:4], -BIG,
                                            op0=ALU.is_lt, op1=ALU.mult)
                masked = tiles(rp, [128, E], "masked")
                for j in ts:
                    for g in range(NG):
                        nc.vector.tensor_scalar_add(
                            masked[j][:, GS * g:GS * (g + 1)],
                            sfc[j][:, GS * g:GS * (g + 1)],
                            pen[j][:, g:g + 1])
                m8 = tiles(sp, [128, 8], "m8")
                for j in ts:
                    nc.vector.max(m8[j][:], masked[j][:])
                i8 = [sp.tile([128, 8], U32, tag="i8", name=f"i8_{ii[j]}")
                      for j in ts]
                for j in ts:
                    nc.vector.max_index(i8[j][:], m8[j][:], masked[j][:])
                i8f = tiles(sp, [128, 8], "i8f")
                for j in ts:
                    nc.vector.tensor_copy(i8f[j][:], i8[j][:])
                # w_raw[k] = m8[k] - bias[i8[k]] (index-matched gather)
                junk = tiles(rp, [128, E], "junk", n=1)
                biasg = tiles(sp, [128, 8], "biasg")
                for j in ts:
                    for k in range(8):
                        nc.vector.scalar_tensor_tensor(
                            junk[0][:], iota_sb[:], i8f[j][:, k:k + 1],
                            bias_sb[:], op0=ALU.is_equal, op1=ALU.mult,
                            accum_out=biasg[j][:, k:k + 1])
                wraw = tiles(sp, [128, 8], "wraw")
                for j in ts:
                    nc.vector.tensor_tensor(wraw[j][:], m8[j][:],
                                            biasg[j][:], op=ALU.subtract)
                ssum = tiles(sp, [128, 1], "ssum")
                for j in ts:
                    nc.vector.tensor_reduce(ssum[j][:], wraw[j][:],
                                            axis=AX.X, op=ALU.add)
                inv = tiles(sp, [128, 1], "inv")
                for j in ts:
                    nc.vector.reciprocal(inv[j][:], ssum[j][:])
                wout = tiles(sp, [128, 8], "wout")
                for j in ts:
                    nc.vector.tensor_scalar(wout[j][:], wraw[j][:],
                                            inv[j][:], 2.5,
                                            op0=ALU.mult, op1=ALU.mult)
                # outputs on the SWDGE ring: keeps their routing-chain wait
                # off the ACT ring (sigmoids) and the SP ring (x prefetch)
                for j in ts:
                    i = ii[j]
                    nc.gpsimd.dma_start(idx_d[128 * i:128 * (i + 1), :],
                                        i8[j][:].bitcast(I32))
                    nc.gpsimd.dma_start(w_d[128 * i:128 * (i + 1), :],
                                        wout[j][:])

            # transpose-back + sigmoid immediately after each block's GEMM;
            # the DVE routing stages for block b-1 are emitted BEFORE block
            # b's GEMM so the DVE queue holds only dependency-ready routing
            # work while the PE grinds through block b's matmuls
            def emit_all():
                held = {}
                for b in range(NB):
                    if b >= 1:
                        emit_route_block(b - 1, held.pop(b - 1))
                    accs = emit_gemm(b)
                    held[b] = emit_back(b, accs)
                emit_route_block(NB - 1, held.pop(NB - 1))

            if repeat == 1:
                emit_all()
            else:
                # benchmarking only: loop the whole body on-device
                with tc.For_i(0, repeat, 1):
                    emit_all()

    nc.compile()
    return nc


_NC_CACHE = {}
_T_FULL = 16384
_N_CORES = 8


def make_maps(hidden_states, weight, e_score_correction_bias):
    """Host prep: bf16-pair split + shard + relayout for the 8 cores."""
    t_core = _T_FULL // _N_CORES
    x = np.asarray(hidden_states, dtype=np.float32).reshape(_T_FULL, H)
    w = np.asarray(weight, dtype=np.float32)
    bias = np.asarray(e_score_correction_bias, dtype=np.float32)

    xh = x.astype(BF16NP)
    xl = (x - xh.astype(np.float32)).astype(BF16NP)

    def relayout_x(a):                       # [T, H] -> [core, p, c*t]
        return np.ascontiguousarray(
            a.reshape(_N_CORES, t_core, NCH, 128).transpose(0, 3, 2, 1)
        ).reshape(_N_CORES, 128, NCH * t_core)

    XH, XL = relayout_x(xh), relayout_x(xl)

    wT = np.ascontiguousarray(w.T)           # [H, E]
    wh = wT.astype(BF16NP)
    wl = (wT - wh.astype(np.float32)).astype(BF16NP)

    def relayout_w(a):                       # [H, E] -> [p, c*e]
        return np.ascontiguousarray(
            a.reshape(NCH, 128, E).transpose(1, 0, 2)
        ).reshape(128, NCH * E)

    base = {
        "wh": relayout_w(wh),
        "wl": relayout_w(wl),
        "bias_b": np.ascontiguousarray(
            np.broadcast_to(bias[None, :], (128, E))),
        "iota_b": np.ascontiguousarray(
            np.broadcast_to(np.arange(E, dtype=np.float32)[None, :],
                            (128, E))),
        "ident": np.eye(128, dtype=np.float32),
    }
    maps = []
    for c in range(_N_CORES):
        m = dict(base)
        m["xh"] = XH[c]
        m["xl"] = XL[c]
        maps.append(m)
    return maps


def kernel(hidden_states, weight, e_score_correction_bias):
    from concourse.bass_utils import run_bass_kernel_spmd

    t_core = _T_FULL // _N_CORES
    maps = make_maps(hidden_states, weight, e_score_correction_bias)

    if "v3" not in _NC_CACHE:
        _NC_CACHE["v3"] = _build(t_core, n_devices=_N_CORES)
    nc = _NC_CACHE["v3"]

    br = run_bass_kernel_spmd(nc, maps, list(range(_N_CORES)))
    idx = np.concatenate(
        [br.results[c]["idx_out"] for c in range(_N_CORES)],
        axis=0).astype(np.int32)
    wout = np.concatenate(
        [br.results[c]["w_out"] for c in range(_N_CORES)],
        axis=0).astype(np.float32)
    return idx, wout
